# revision 49
# baseline (speedup 1.0000x reference)
"""Bass/Trainium2 kernel for nn_CapsuleLayer (dynamic routing capsule layer).

Reference computation:
    inputs: [B=32, J=2048, I=64], W: [K=32, J=2048, D=32, I=64]
    hat[b,j,d,k] = sum_i inputs[b,j,i] * W[k,j,d,i]
    3 routing iterations (softmax over K), output = squash(s_2)  [B, K, D]

Sharding: J (input capsules) split 8 ways -> JL = 256 per core.
Per-iteration s[b,k,d] partial sums need a 64KB fp16 AllReduce.

v2 design (vs the DVE-bound v1):
  - Pass A runs W-STATIONARY: lhsT = W chunk [(jp,i)=128, dk=128] (gets FWL),
    rhs = block-diag x station [128, 64] -> hat lands in "hat3" layout
    [p=(dm4,k), chunk c, b, j] with d = 4c + dm4.  PE cost halves and the
    (d,k) axis lands in PARTITIONS.
  - u[b,k,j] = sum_d O[b,k,d] hat: done ON PE with delta-masked stations
    osta[p,k'] = (p%32==k') * O[p-layout], accumulated over the 8 dk-chunks.
    4 batches b share one PSUM tile via tile_position col groups.
  - softmax over k stays in k-in-partition form: exp on ScalarE (shifted to
    keep fp16 range), Z = sum_k via a block-ones PE matmul, 1/Z on DVE.
  - Only c (x) hat multiply + the j-reduction tree remain on DVE/Pool.
  - s0 = sum_j hat computed with DVE reduce_sum under the pass-A DMA shadow.
  - A tiny warm-up AllReduce at t=0 absorbs the collective rendezvous cost.
"""

import os
import sys
import numpy as np

import concourse.bass as bass
import concourse.mybir as mybir
import concourse.tile as tile
from concourse import bacc
from concourse import bass_utils

AF = mybir.ActivationFunctionType
ALU = mybir.AluOpType
F16 = mybir.dt.float16
F32 = mybir.dt.float32

EPS = 1e-07
N_CORES = 8
B = 32          # batch
J = 2048        # input capsules (total)
I = 64          # input capsule dim
K = 32          # output capsules
D = 32          # output capsule dim
JL = J // N_CORES          # 256 local input capsules
NPAIR = JL // 2            # 128 station pairs
DK = D * K                 # 1024
NC = DK // 128             # 8 dk-chunks of 128
EXP_SHIFT = -5.0           # softmax logit shift so exp() fits fp16 range


def build_program():
    """Build the SPMD bass program (same program on all 8 cores)."""
    nc = bacc.Bacc("TRN2", target_bir_lowering=False, debug=False,
                   enable_asserts=False, num_devices=N_CORES)

    xs = nc.dram_tensor("xs", [128, NPAIR, I], F16, kind="ExternalInput").ap()
    wt = nc.dram_tensor("wt", [NPAIR, 128, DK], F16, kind="ExternalInput").ap()
    mask = nc.dram_tensor("mask", [128, K], F16, kind="ExternalInput").ap()
    zsta = nc.dram_tensor("zsta", [128, 128], F16, kind="ExternalInput").ap()
    rep32 = nc.dram_tensor("rep32", [K, 128], F32, kind="ExternalInput").ap()
    crep = nc.dram_tensor("crep", [128, 4, 128], F16,
                          kind="ExternalInput").ap()
    out_d = nc.dram_tensor("out", [128, NC * B], F32,
                           kind="ExternalOutput").ap()

    with tile.TileContext(nc) as tc:
        _emit(tc, xs, wt, mask, zsta, rep32, crep, out_d)
    nc.compile()
    return nc


def _emit(tc, xs_d, wt_d, mask_d, zsta_d, rep32_d, crep_d, out_d):
    nc = tc.nc
    with (
        tc.tile_pool(name="hat", bufs=1) as hat_pool,
        tc.tile_pool(name="cst", bufs=1) as const_pool,
        tc.tile_pool(name="wld", bufs=7) as w_pool,
        tc.tile_pool(name="xld", bufs=2) as x_pool,
        tc.tile_pool(name="chp", bufs=2) as ch_pool,
        tc.tile_pool(name="ep", bufs=2) as e_pool,
        tc.tile_pool(name="rzp", bufs=1) as rz_pool,
        tc.tile_pool(name="ckp", bufs=2) as ck_pool,
        tc.tile_pool(name="sm", bufs=1) as small_pool,
        tc.tile_pool(name="s16p", bufs=1) as s16_pool,
        tc.tile_pool(name="psA", bufs=2, space="PSUM") as psA_pool,
        tc.tile_pool(name="psU", bufs=2, space="PSUM") as psU_pool,
        tc.tile_pool(name="psZ", bufs=1, space="PSUM") as psZ_pool,
        tc.tile_pool(name="psC", bufs=2, space="PSUM") as psC_pool,
        tc.tile_pool(name="dram", bufs=8, space="DRAM") as dram_pool,
    ):
        # ---- constants ----
        mask_sb = const_pool.tile([128, K], F16, tag="mask")
        nc.sync.dma_start(mask_sb[:], mask_d)
        zsta_sb = const_pool.tile([128, 128], F16, tag="zsta")
        nc.sync.dma_start(zsta_sb[:], zsta_d)
        rep32_sb = const_pool.tile([K, 128], F32, tag="rep32")
        nc.sync.dma_start(rep32_sb[:], rep32_d)
        crep_sb = const_pool.tile([128, 4, 128], F16, tag="crep")
        nc.sync.dma_start(crep_sb[:], crep_d)
        ebias = const_pool.tile([128, 1], F32, tag="ebias")
        nc.vector.memset(ebias[:], EXP_SHIFT)
        mask32 = const_pool.tile([128, K], F32, tag="mask32")
        nc.vector.tensor_copy(mask32[:], mask_sb[:])

        # warm up the collective path early (absorbs CC rendezvous latency)
        warm_in = dram_pool.tile([128, K], F16, name="warm_in")
        warm_out = dram_pool.tile([128, K], F16, name="warm_out")
        nc.sync.dma_start(warm_in[:], mask_d)
        nc.gpsimd.collective_compute(
            "AllReduce", ALU.add,
            replica_groups=[list(range(N_CORES))],
            ins=[warm_in.opt()],
            outs=[warm_out.opt()],
        )

        # persistent tensors
        # hat3[p=(dm4,k), c, b, j] = hat[b, j, dk=c*128+p]  (d = 4c + dm4)
        hat3 = hat_pool.tile([128, NC, B, JL], F16, tag="hat")
        o_acc = const_pool.tile([128, NC, B], F32, tag="oacc")
        o_acc16 = const_pool.tile([128, NC, B], F16, tag="oacc16")
        s0h = const_pool.tile([128, NC, B, 2], F32, tag="s0h")
        acsc = const_pool.tile([128, JL], F16, tag="acsc")  # Act accum scratch

        # ---- Pass A: hat = x @ W (W stationary), s0 = sum_j hat ----
        # Act does all psum->sbuf copies (GPSIMD can't read PSUM);
        # DVE does the s0 reduces under the DMA shadow.
        for pr in range(NPAIR):
            if pr % 8 == 0:
                xs_t = x_pool.tile([128, 8, I], F16, tag="xs",
                                   name=f"xs_{pr}")
                nc.sync.dma_start(xs_t[:], xs_d[:, pr:pr + 8, :])
            if pr % 2 == 0:
                wq2 = w_pool.tile([128, 2, DK], F16, tag="w", name=f"w_{pr}")
                weng = nc.sync if (pr // 2) % 2 == 0 else nc.gpsimd
                weng.dma_start(
                    wq2[:], wt_d[pr:pr + 2].rearrange("q p f -> p q f"))
            ps = psA_pool.tile([128, NC, 2 * B], F32, tag="psA",
                               name=f"psA_{pr}")
            for c in range(NC):
                nc.tensor.matmul(
                    ps[:, c, :],
                    lhsT=wq2[:, pr % 2, c * 128:(c + 1) * 128],
                    rhs=xs_t[:, pr % 8, :],
                    start=True, stop=True,
                )
            csrc = ps.rearrange("p c (b q) -> p c b q", q=2)
            dst = hat3[:, :, :, 2 * pr:2 * pr + 2]
            nc.scalar.copy(dst, csrc)
            # s0 partial reduces once each j-half is complete
            if pr == NPAIR // 2 - 1 or pr == NPAIR - 1:
                h = 0 if pr < NPAIR // 2 else 1
                for bb in range(16):
                    bs = 2 * bb
                    nc.vector.reduce_sum(
                        s0h[:, :, bs:bs + 2, h:h + 1],
                        hat3[:, :, bs:bs + 2, 128 * h:128 * (h + 1)],
                        axis=mybir.AxisListType.X,
                    )

        HB = B // 2
        s16h = [s16_pool.tile([128, NC * HB], F16, tag=f"s16h{h}",
                              name=f"s16_0_{h}") for h in (0, 1)]
        for h in (0, 1):
            nc.vector.tensor_tensor(
                s16h[h].rearrange("p (c b) -> p c b", c=NC),
                s0h[:, :, HB * h:HB * (h + 1), 0],
                s0h[:, :, HB * h:HB * (h + 1), 1], ALU.add,
            )

        # ---- routing iterations, pipelined over b-halves ----
        # Each iteration's AllReduce is split into two 16-batch halves; a
        # half's collective overlaps the other half's compute, and the next
        # iteration's AR launches as soon as blocks 0-3 finish their tails.
        s16b = [[None] * 2 for _ in range(3)]
        for r in range(3):
            for h in (0, 1):
                ar_in = dram_pool.tile([128, NC * HB], F16,
                                       name=f"ar_in{r}_{h}")
                ar_out = dram_pool.tile([128, NC * HB], F16,
                                        name=f"ar_out{r}_{h}")
                nc.sync.dma_start(ar_in[:], s16h[h][:])
                nc.gpsimd.collective_compute(
                    "AllReduce", ALU.add,
                    replica_groups=[list(range(N_CORES))],
                    ins=[ar_in.opt()],
                    outs=[ar_out.opt()],
                )
                sb_t = s16_pool.tile([128, NC * HB], F16, tag=f"s16b{h}",
                                     name=f"s16b{r}_{h}")
                nc.sync.dma_start(sb_t[:], ar_out[:])
                s16b[r][h] = sb_t

            if r < 2:
                s16h = [s16_pool.tile([128, NC * HB], F16, tag=f"s16h{h}",
                                      name=f"s16_{r + 1}_{h}")
                        for h in (0, 1)]
                s32 = small_pool.tile([128, NC, B], F32, tag="s32",
                                      name=f"s32_{r}")

            def emit_squash(h, r=r):
                """squash + o for b-half h; returns nothing (writes o_acc16
                half or the output)."""
                bsl = slice(HB * h, HB * (h + 1))
                s_full = small_pool.tile([128, NC, HB], F32, tag=f"sf{h}",
                                         name=f"sfull{r}_{h}")
                sfv = s_full.rearrange("p c b -> p (c b)")
                if r == 0:
                    nc.vector.tensor_scalar_mul(sfv, s16b[r][h][:], 1.0 / K)
                else:
                    nc.vector.tensor_copy(sfv, s16b[r][h][:])
                sq = small_pool.tile([128, NC, HB], F32, tag=f"sq{h}",
                                     name=f"sq{r}_{h}")
                nc.scalar.square(sq[:], s_full[:])
                t4 = small_pool.tile([128, 4, HB], F32, tag=f"t4{h}",
                                     name=f"t4_{r}_{h}")
                nc.vector.tensor_tensor(t4[:], sq[:, 0:4, :], sq[:, 4:8, :],
                                        ALU.add)
                t2 = small_pool.tile([128, 2, HB], F32, tag=f"t2{h}",
                                     name=f"t2_{r}_{h}")
                nc.vector.tensor_tensor(t2[:], t4[:, 0:2, :], t4[:, 2:4, :],
                                        ALU.add)
                t1 = small_pool.tile([128, HB], F32, tag=f"t1{h}",
                                     name=f"t1_{r}_{h}")
                nc.vector.tensor_tensor(t1[:], t2[:, 0, :], t2[:, 1, :],
                                        ALU.add)
                sqz = psZ_pool.tile([128, JL], F32, tag="sq", bufs=1,
                                    name=f"sqz{r}_{h}")
                nc.tensor.matmul(sqz[0:K, 0:HB], lhsT=mask32[:], rhs=t1[:],
                                 start=True, stop=True)
                s2e = small_pool.tile([K, HB], F32, tag=f"s2e{h}",
                                      name=f"s2e{r}_{h}")
                nc.vector.tensor_scalar_add(s2e[:], sqz[0:K, 0:HB], EPS)
                rt = small_pool.tile([K, HB], F32, tag=f"rt{h}",
                                     name=f"rt{r}_{h}")
                nc.scalar.sqrt(rt[:], s2e[:])
                den = small_pool.tile([K, HB], F32, tag=f"den{h}",
                                      name=f"den{r}_{h}")
                nc.vector.scalar_tensor_tensor(den[:], sqz[0:K, 0:HB], 1.0,
                                               rt[:], ALU.add, ALU.mult)
                rden = small_pool.tile([K, HB], F32, tag=f"rden{h}",
                                       name=f"rden{r}_{h}")
                nc.vector.reciprocal_approx_fast(rden[:], den[:])
                scl = small_pool.tile([K, HB], F32, tag=f"scl{h}",
                                      name=f"scl{r}_{h}")
                nc.vector.tensor_tensor(scl[:], sqz[0:K, 0:HB], rden[:],
                                        ALU.mult)
                sclp = psZ_pool.tile([128, JL], F32, tag="sq", bufs=1,
                                     name=f"sclp{r}_{h}")
                nc.tensor.matmul(sclp[:, 0:HB], lhsT=rep32_sb[:], rhs=scl[:],
                                 start=True, stop=True)
                scl128 = small_pool.tile([128, HB], F32, tag=f"sc128{h}",
                                         name=f"sc128_{r}_{h}")
                nc.scalar.copy(scl128[:], sclp[:, 0:HB])
                o_r = small_pool.tile([128, NC, HB], F32, tag=f"or{h}",
                                      name=f"or{r}_{h}")
                nc.vector.tensor_tensor(
                    o_r[:],
                    s_full[:],
                    scl128[:, None, :].to_broadcast([128, NC, HB]),
                    ALU.mult,
                )
                if r == 2:
                    nc.sync.dma_start(
                        out_d.rearrange("p (c b) -> p c b", c=NC)[:, :, bsl],
                        o_r[:])
                    return
                if r == 0:
                    nc.vector.tensor_copy(o_acc[:, :, bsl], o_r[:])
                else:
                    nc.vector.tensor_add(o_acc[:, :, bsl], o_acc[:, :, bsl],
                                         o_r[:])
                nc.scalar.copy(o_acc16[:, :, bsl], o_acc[:, :, bsl])

            emit_squash(0)
            if r == 2:
                emit_squash(1)
                break

            # routing pass over 4-b blocks, software-pipelined; block k's
            # head is emitted before block k-2's tail.
            A_SET = {1, 4, 7, 9, 12, 15}   # sub-blocks reduced on ScalarE
            pends = []

            def emit_tail(cek, b0, base, r=r, s32=s32, s16h=s16h):
                for sb in range(2):
                    bs = b0 + 2 * sb
                    ch = ch_pool.tile([128, NC, 2, JL], F16, tag="ch",
                                      name=f"ch{r}_{bs}")
                    nc.vector.tensor_tensor(
                        ch[:],
                        hat3[:, :, bs:bs + 2, :],
                        cek[:, None, 2 * sb:2 * sb + 2, :].to_broadcast(
                            [128, NC, 2, JL]),
                        ALU.mult,
                    )
                    if (base + sb) in A_SET:
                        for c in range(NC):
                            for s in range(2):
                                nc.scalar.activation(
                                    acsc[:], ch[:, c, s, :], AF.Copy,
                                    accum_out=s32[:, c, bs + s:bs + s + 1])
                    else:
                        nc.vector.reduce_sum(
                            s32[:, :, bs:bs + 2],
                            ch[:, :, :, 0:JL],
                            axis=mybir.AxisListType.X)

            def stage_half(h, r=r, s32=s32, s16h=s16h):
                nc.scalar.copy(
                    s16h[h].rearrange("p (c b) -> p c b", c=NC),
                    s32[:, :, HB * h:HB * (h + 1)])

            for bb4 in range(8):
                if bb4 == 4:
                    emit_squash(1)
                b0 = 4 * bb4
                # delta-masked stations for this block's 4 b's (on Pool):
                # osta[p, c, g, k'] = (p%32==k') * O_acc[p, c, b0+g]
                osta = ck_pool.tile([128, NC, 4, K], F16, tag="osta", bufs=2,
                                    name=f"osta{r}_{bb4}")
                nc.gpsimd.tensor_tensor(
                    osta[:],
                    mask_sb[:, None, None, :].to_broadcast([128, NC, 4, K]),
                    o_acc16[:, :, b0:b0 + 4, None].to_broadcast(
                        [128, NC, 4, K]),
                    ALU.mult,
                )
                u_ps = psU_pool.tile([128, JL], F32, tag="ups",
                                     name=f"ups{r}_{bb4}")
                for c in range(NC):
                    for g in range(4):
                        nc.tensor.matmul(
                            u_ps[32 * g:32 * (g + 1), :],
                            lhsT=osta[:, c, g, :],
                            rhs=hat3[:, c, b0 + g, :],
                            start=(c == 0), stop=(c == NC - 1),
                            tile_position=(0, 32 * g),
                            skip_group_check=True,
                        )
                e16 = e_pool.tile([128, JL], F16, tag="e16",
                                  name=f"e{r}_{bb4}")
                nc.scalar.activation(e16[:], u_ps[:], AF.Exp, bias=ebias[:])
                z_ps = psZ_pool.tile([128, JL], F32, tag="zps",
                                     name=f"z{r}_{bb4}")
                nc.tensor.matmul(z_ps[:], lhsT=zsta_sb[:], rhs=e16[:],
                                 start=True, stop=True)
                rz32 = rz_pool.tile([128, JL], F32, tag="rz32",
                                    name=f"rz32_{r}_{bb4}")
                nc.vector.reciprocal_approx_fast(rz32[:], z_ps[:])
                # softmax weights c = e/Z, k-in-partition layout
                c_k4 = ck_pool.tile([128, JL], F16, tag="ck", bufs=2,
                                    name=f"ck{r}_{bb4}")
                nc.vector.tensor_tensor(c_k4[:], e16[:], rz32[:], ALU.mult)
                # replicate each b's [k, j] block to all 128 partitions (PE)
                cek = ck_pool.tile([128, 4, JL], F16, tag="cexp", bufs=3,
                                   name=f"cek{r}_{bb4}")
                for g in range(4):
                    ce = psC_pool.tile([128, JL], F32, tag="ce",
                                       name=f"ce{r}_{bb4}_{g}")
                    nc.tensor.matmul(ce[:], lhsT=crep_sb[:, g, :],
                                     rhs=c_k4[:], start=True, stop=True)
                    nc.scalar.copy(cek[:, g, :], ce[:])
                pends.append((cek, b0, 2 * bb4))
                if len(pends) > 2:
                    emit_tail(*pends.pop(0))
                    if pends[0][1] == 16:   # blocks 0-3 tails all emitted
                        stage_half(0)
            for p_ in pends:
                emit_tail(*p_)
            stage_half(1)

def pack_inputs(inputs, W):
    """Host-side shard + layout pack. Returns in_maps (one dict per core)."""
    mask = np.zeros((128, K), np.float16)
    mask[np.arange(128), np.arange(128) % K] = 1.0
    zsta = np.kron(np.eye(4, dtype=np.float16),
                   np.ones((32, 32), np.float16))
    # rep32[k, m] = (m%32 == k): replicates a [32, .] tile to 128 partitions
    rep32 = np.zeros((K, 128), np.float32)
    rep32[np.arange(128) % K, np.arange(128)] = 1.0
    # crep[p', g, m] = (p'//32 == g) & (p'%32 == m%32): selects b-group g's
    # [k, j] block and replicates it across the 4 dm4 partition groups
    crep = np.zeros((128, 4, 128), np.float16)
    pp = np.arange(128)
    for g in range(4):
        sel = (pp // 32 == g)
        for m in range(128):
            crep[sel & (pp % 32 == m % 32), g, m] = 1.0

    in_maps = []
    for c in range(N_CORES):
        jsl = slice(c * JL, (c + 1) * JL)
        # W: [K, J, D, I] -> [JL, I, D, K] -> [pair, (jp,i), (d,k)] fp16
        wc = np.ascontiguousarray(
            W[:, jsl].transpose(1, 3, 2, 0), dtype=np.float16
        )  # [JL, I, D, K]
        wt = wc.reshape(NPAIR, 2 * I, DK)

        # x stations: xs[p=(jp,i), pair, col=2b+jp] block-diag, partition-major
        xc = inputs[:, jsl, :]  # [B, JL, I]
        xt = np.ascontiguousarray(xc.transpose(1, 2, 0)).astype(np.float16)
        xs = np.zeros((NPAIR, 128, I), np.float16)
        xs[:, 0:I, 0::2] = xt[0::2]      # jp=0 rows, even cols
        xs[:, I:128, 1::2] = xt[1::2]    # jp=1 rows, odd cols
        xs2 = np.ascontiguousarray(xs.transpose(1, 0, 2))  # [128, NPAIR, I]
        in_maps.append({"xs": xs2, "wt": wt, "mask": mask, "zsta": zsta,
                        "rep32": rep32, "crep": crep})
    return in_maps


_CACHED_NC = None


def _install_ntff_hook():
    """Provide antenv.axon_hooks.get_axon_ntff_profile_hook when the agent
    image lacks it, by driving the injected libaxon_pjrt.so directly
    (mirrors trn_agent_boot._ntff_profile_via_ctypes)."""
    import types
    import ctypes
    import contextlib
    try:
        from antenv.axon_hooks import get_axon_ntff_profile_hook  # noqa: F401
        return True
    except ImportError:
        pass
    so_path = "/opt/axon/libaxon_pjrt.so"
    if not os.path.exists(so_path):
        return False
    lib = ctypes.CDLL(so_path)
    if not hasattr(lib, "axon_start_nrt_profile"):
        return False
    lib.axon_start_nrt_profile.argtypes = [
        ctypes.POINTER(ctypes.c_int64), ctypes.c_size_t]
    lib.axon_start_nrt_profile.restype = ctypes.c_int64
    lib.axon_stop_nrt_profile.argtypes = [ctypes.c_char_p]
    lib.axon_stop_nrt_profile.restype = ctypes.c_int64

    @contextlib.contextmanager
    def _hook(output_dir, device_ids):
        import jax
        jax.devices()
        if device_ids:
            ids = (ctypes.c_int64 * len(device_ids))(*device_ids)
            rc = lib.axon_start_nrt_profile(ids, len(device_ids))
        else:
            rc = lib.axon_start_nrt_profile(None, 0)
        if rc != 0:
            raise RuntimeError(f"axon_start_nrt_profile rc={rc}")
        try:
            yield
        finally:
            n = lib.axon_stop_nrt_profile(str(output_dir).encode())
            if n < 0:
                raise RuntimeError(f"axon_stop_nrt_profile rc={n}")

    import antenv
    mod = types.ModuleType("antenv.axon_hooks")
    mod.get_axon_ntff_profile_hook = lambda: _hook
    mod.set_axon_ntff_profile_hook = lambda h: None
    sys.modules["antenv.axon_hooks"] = mod
    antenv.axon_hooks = mod
    return True


def kernel(inputs, W):
    global _CACHED_NC
    inputs = np.asarray(inputs)
    W = np.asarray(W)
    if _CACHED_NC is None:
        _CACHED_NC = build_program()
    nc = _CACHED_NC
    in_maps = pack_inputs(inputs, W)
    trace = bool(int(os.environ.get("CAPS_TRACE", "0")))
    if trace:
        trace = _install_ntff_hook()
    res = bass_utils.run_bass_kernel_spmd(
        nc, in_maps, core_ids=list(range(N_CORES)), trace=trace,
    )
    kernel.last_results = res
    if trace and res.exec_time_ns is not None:
        print(f"HW exec time: {res.exec_time_ns} ns", file=sys.stderr)
        kernel.last_exec_time_ns = res.exec_time_ns
    out = res.results[0]["out"]  # [128 p=(dm4,k), NC*B] fp32 device layout
    a = out.reshape(4, K, NC, B)         # [dm4, k, c, b]; d = 4c + dm4
    return np.ascontiguousarray(
        a.transpose(3, 1, 2, 0).reshape(B, K, D)
    ).astype(np.float32)


kernel.last_exec_time_ns = None
kernel.last_results = None
